# revision 7
# baseline (speedup 1.0000x reference)
"""Distributed Trainium2 (8 NeuronCore) kernel for nn_Attention_54382875902030.

Reference computation (B=2, N=2048, D=2048, H=16, DH=128):
    q,k,v = split_heads(x @ Wq/Wk/Wv);  RoPE(q), RoPE(k)
    out   = softmax(q k^T / sqrt(DH)) v
    out  *= sigmoid(x @ Wg + bg)  (per-head gate)
    return merge_heads(out) @ Wo

Sharding: 8 cores = 2 batch groups x 4 sequence blocks (512 query rows each).
Each core projects q/k/v/gate for its own 512 rows (all 16 heads), then
AllGathers k^T and v within its batch group (replica groups [[0..3],[4..7]]),
runs full non-causal attention for its 512 query rows over all 2048 keys,
and computes its block of the output projection locally (full Wo). The
output is a disjoint row-slice per core -> host-side gather is a pure
concatenation (no reduction).

All matmuls run in bf16 (fp32 PSUM accumulation). Attention is computed in
the transposed layout S^T[k, q] = k . q^T so that:
  - exp(SCALE * S^T) needs no per-row bias (inputs are ~N(0,1) -> scores
    bounded well inside fp32/bf16 exp range, max-subtraction skipped),
  - softmax denominators come from a ones-vector matmul (partition-axis sum),
  - P^T feeds the PV matmul directly (no transposes anywhere in the kernel).
"""

import numpy as np
import ml_dtypes

bf16 = ml_dtypes.bfloat16

B, N, D = 2, 2048, 2048
H, DH = 16, 128
NB = N // 4          # 512 local query rows per core
DC = D // 128        # 16 contraction chunks
SCALE = DH ** -0.5
NCORES = 8
GROUPS = [[0, 1, 2, 3], [4, 5, 6, 7]]

_CACHE = {}


def _build():
    import concourse.bacc as bacc
    import concourse.tile as tile
    import concourse.mybir as mybir

    F32 = mybir.dt.float32
    BF = mybir.dt.bfloat16
    ACT = mybir.ActivationFunctionType

    nc = bacc.Bacc(None, target_bir_lowering=False, num_devices=NCORES)

    # ---- parameters (per-core shards / replicated weights) ----
    xt = nc.declare_dram_parameter("xt", [D, NB], BF, isOutput=False)       # x rows, transposed
    cos = nc.declare_dram_parameter("cos", [DH, NB], BF, isOutput=False)    # cos(rotary).T local
    sins = nc.declare_dram_parameter("sins", [DH, NB], BF, isOutput=False)  # signed sin(rotary).T local
    wq = nc.declare_dram_parameter("wq", [D, D], BF, isOutput=False)
    wk = nc.declare_dram_parameter("wk", [D, D], BF, isOutput=False)
    wv = nc.declare_dram_parameter("wv", [D, D], BF, isOutput=False)
    wg = nc.declare_dram_parameter("wg", [D, H], BF, isOutput=False)
    bg = nc.declare_dram_parameter("bg", [H, 1], F32, isOutput=False)
    wo = nc.declare_dram_parameter("wo", [D, D], BF, isOutput=False)
    out = nc.declare_dram_parameter("out", [NB, D], F32, isOutput=True)

    ones_c = nc.inline_tensor(np.ones((128, 1), bf16), name="ones_c")

    with tile.TileContext(nc) as tc:
        with (
            tc.tile_pool(name="dram", bufs=1, space="DRAM") as dram,
            tc.tile_pool(name="persist", bufs=1) as persist,
            tc.tile_pool(name="qt_pool", bufs=1) as qt_pool,
            tc.tile_pool(name="og_pool", bufs=1) as og_pool,
        ):
            # collective bounce buffers
            k_in = dram.tile([H * DH, NB], BF)        # [2048, 512] packed k^T (head-major)
            k_out = dram.tile([4 * H * DH, NB], BF)   # [8192, 512] rank-major gather
            v_in = dram.tile([NB, H * DH], BF)        # [512, 2048] v natural rows
            v_out = dram.tile([4 * NB, H * DH], BF)   # [2048, 2048] full v natural

            # ---- constants & activations resident in SBUF ----
            xt_sb = persist.tile([128, DC * NB], BF)          # x^T, free=(dchunk, n)
            nc.sync.dma_start(
                xt_sb[:].rearrange("p (c n) -> p c n", c=DC),
                xt.ap().rearrange("(c p) n -> p c n", p=128),
            )
            cos_sb = persist.tile([128, NB], BF)
            sins_sb = persist.tile([128, NB], BF)
            nc.sync.dma_start(cos_sb[:], cos.ap())
            nc.sync.dma_start(sins_sb[:], sins.ap())
            bg_sb = persist.tile([H, 1], F32)
            nc.sync.dma_start(bg_sb[:], bg.ap())
            ones_sb = persist.tile([128, 1], BF)
            nc.sync.dma_start(ones_sb[:], ones_c.ap())
            gate_sb = persist.tile([H, NB], BF)
            gate_flat = persist.tile([1, H * NB], BF)
            wo_sb = persist.tile([128, H * D], BF)             # Wo, free=(head, cols)

            wk_r = wk.ap().rearrange("(c p) m -> p c m", p=128)
            wq_r = wq.ap().rearrange("(c p) m -> p c m", p=128)
            wv_r = wv.ap().rearrange("(c p) m -> p c m", p=128)

            def qk_proj_head(w_r, h, wpool, psum, evac, dst_sb):
                """project head h -> RoPE -> dst_sb [128(dh), NB] bf16."""
                w_sb = wpool.tile([128, DC * DH], BF, tag="wqk")
                nc.sync.dma_start(
                    w_sb[:].rearrange("p (c m) -> p c m", c=DC),
                    w_r[:, :, h * DH:(h + 1) * DH],
                )
                pk = psum.tile([128, NB], F32, tag="pacc")
                for dc in range(DC):
                    nc.tensor.matmul(
                        pk[:],
                        w_sb[:, dc * DH:(dc + 1) * DH],
                        xt_sb[:, dc * NB:(dc + 1) * NB],
                        start=(dc == 0),
                        stop=(dc == DC - 1),
                    )
                # RoPE: dst = t*cos + rot(t)*sins   (sins rows 0:64 pre-negated)
                tf = evac.tile([128, NB], BF, tag="tf")
                nc.scalar.activation(tf[:], pk[:], ACT.Copy)
                t1 = evac.tile([128, NB], BF, tag="t1")
                nc.vector.tensor_mul(t1[:], tf[:], cos_sb[:])
                # rotate-half via address-based SBUF->SBUF DMA (engines can't
                # mix SBUF base partitions within one tensor_tensor)
                rot = evac.tile([128, NB], BF, tag="rot")
                nc.sync.dma_start(rot[0:64, :], tf[64:128, :])
                nc.sync.dma_start(rot[64:128, :], tf[0:64, :])
                t2 = evac.tile([128, NB], BF, tag="t2")
                nc.vector.tensor_mul(t2[:], rot[:], sins_sb[:])
                nc.vector.tensor_add(dst_sb[:], t1[:], t2[:])

            with (
                tc.tile_pool(name="wpool", bufs=2) as wpool,
                tc.tile_pool(name="psum_proj", bufs=2, space="PSUM") as psum,
                tc.tile_pool(name="evac", bufs=2) as evac,
                tc.tile_pool(name="kt_pool", bufs=3) as kt_pool,
            ):
                # ---- k projection + RoPE + pack for AllGather ----
                for h in range(H):
                    kt = kt_pool.tile([128, NB], BF, tag="kt")
                    qk_proj_head(wk_r, h, wpool, psum, evac, kt)
                    nc.sync.dma_start(k_in[h * DH:(h + 1) * DH, :], kt[:])
                nc.gpsimd.collective_compute(
                    "AllGather",
                    mybir.AluOpType.bypass,
                    replica_groups=GROUPS,
                    ins=[k_in.opt()],
                    outs=[k_out.opt()],
                )

                # ---- v projection (natural layout) + pack for AllGather ----
                v_loc = persist.tile([128, 4 * 4 * NB], BF)   # free=(jn, cc, n)
                for cc in range(4):
                    wv_sb = wpool.tile([128, DC * NB], BF, tag="wv")
                    nc.sync.dma_start(
                        wv_sb[:].rearrange("p (c m) -> p c m", c=DC),
                        wv_r[:, :, cc * NB:(cc + 1) * NB],
                    )
                    for jn in range(4):
                        pv = psum.tile([128, NB], F32, tag="pacc")
                        for dc in range(DC):
                            nc.tensor.matmul(
                                pv[:],
                                xt_sb[:, dc * NB + jn * 128:dc * NB + (jn + 1) * 128],
                                wv_sb[:, dc * NB:(dc + 1) * NB],
                                start=(dc == 0),
                                stop=(dc == DC - 1),
                            )
                        nc.scalar.activation(
                            v_loc[:, (jn * 4 + cc) * NB:(jn * 4 + cc + 1) * NB],
                            pv[:],
                            ACT.Copy,
                        )
                nc.sync.dma_start(
                    v_in.rearrange("(jn p) (cc n) -> p jn cc n", p=128, cc=4),
                    v_loc[:].rearrange("p (jn cc n) -> p jn cc n", jn=4, cc=4),
                )
                nc.gpsimd.collective_compute(
                    "AllGather",
                    mybir.AluOpType.bypass,
                    replica_groups=GROUPS,
                    ins=[v_in.opt()],
                    outs=[v_out.opt()],
                )

                # ---- q projection + RoPE (kept in SBUF) ----
                qts = []
                for h in range(H):
                    qt = qt_pool.tile([128, NB], BF, tag=f"qt{h}")
                    qk_proj_head(wq_r, h, wpool, psum, evac, qt)
                    qts.append(qt)

                # ---- gate: sigmoid(Wg^T x^T + bg) -> [H, NB] ----
                wg_sb = persist.tile([128, DC * H], BF)
                nc.sync.dma_start(
                    wg_sb[:].rearrange("p (c g) -> p c g", c=DC),
                    wg.ap().rearrange("(c p) g -> p c g", p=128),
                )
                pg = psum.tile([H, NB], F32, tag="pg")
                for dc in range(DC):
                    nc.tensor.matmul(
                        pg[:],
                        wg_sb[:, dc * H:(dc + 1) * H],
                        xt_sb[:, dc * NB:(dc + 1) * NB],
                        start=(dc == 0),
                        stop=(dc == DC - 1),
                    )
                nc.scalar.activation(gate_sb[:], pg[:], ACT.Sigmoid, bias=bg_sb[:])
                # reshape gate to [1, H*NB] on partition 0 (engine ops can't
                # read partition bases that aren't 0/32/64; DMA is address-based
                # so per-row SBUF->SBUF copies are fine)
                for h in range(H):
                    nc.sync.dma_start(
                        gate_flat[0:1, h * NB:(h + 1) * NB], gate_sb[h:h + 1, :]
                    )

            # Wo preload (overlaps attention)
            nc.sync.dma_start(
                wo_sb[:].rearrange("p (h m) -> p h m", h=H),
                wo.ap().rearrange("(h p) m -> p h m", p=128),
            )

            k_out_r = k_out.rearrange("(r h p) n -> p h r n", r=4, h=H, p=128)
            v_out_r = v_out.rearrange("(ck p) (hh dh) -> p ck hh dh", p=128, hh=H)

            # ---- attention (16 heads x [2048 k, 512 q]) ----
            KC = N // 128  # 16 key chunks
            ogs = []
            with (
                tc.tile_pool(name="kv_io", bufs=2) as kv_io,
                tc.tile_pool(name="ps_s", bufs=2, space="PSUM") as ps_s,
                tc.tile_pool(name="ps_o", bufs=2, space="PSUM") as ps_o,
                tc.tile_pool(name="ps_l", bufs=2, space="PSUM") as ps_l,
                tc.tile_pool(name="p_pool", bufs=3) as p_pool,
                tc.tile_pool(name="smalls", bufs=2) as smalls,
            ):
                for h in range(H):
                    kt_g = kv_io.tile([128, N], BF, tag="ktg")
                    nc.sync.dma_start(
                        kt_g[:].rearrange("p (r n) -> p r n", r=4),
                        k_out_r[:, h],
                    )
                    v_g = kv_io.tile([128, KC * DH], BF, tag="vg")
                    nc.sync.dma_start(
                        v_g[:].rearrange("p (ck dh) -> p ck dh", ck=KC),
                        v_out_r[:, :, h],
                    )
                    po = ps_o.tile([128, NB], F32, tag="po")
                    pl = ps_l.tile([1, NB], F32, tag="pl")
                    for pr in range(KC // 2):
                        ps = ps_s.tile([128, 2 * NB], F32, tag="ps")
                        for half in range(2):
                            ik = 2 * pr + half
                            nc.tensor.matmul(
                                ps[:, half * NB:(half + 1) * NB],
                                kt_g[:, ik * 128:(ik + 1) * 128],
                                qts[h][:],
                                start=True,
                                stop=True,
                            )
                        pexp = p_pool.tile([128, 2 * NB], BF, tag="pexp")
                        nc.scalar.activation(pexp[:], ps[:], ACT.Exp, scale=SCALE)
                        for half in range(2):
                            ik = 2 * pr + half
                            nc.tensor.matmul(
                                po[:],
                                v_g[:, ik * DH:(ik + 1) * DH],
                                pexp[:, half * NB:(half + 1) * NB],
                                start=(ik == 0),
                                stop=(ik == KC - 1),
                            )
                            nc.tensor.matmul(
                                pl[:],
                                ones_sb[:],
                                pexp[:, half * NB:(half + 1) * NB],
                                start=(ik == 0),
                                stop=(ik == KC - 1),
                            )
                    # c = sigmoid-gate / softmax-sum, broadcast over partitions
                    lr = smalls.tile([1, NB], F32, tag="lr")
                    nc.vector.reciprocal(lr[:], pl[:])
                    cs = smalls.tile([1, NB], BF, tag="cs")
                    nc.vector.tensor_mul(cs[:], lr[:], gate_flat[0:1, h * NB:(h + 1) * NB])
                    cb = smalls.tile([128, NB], BF, tag="cb")
                    nc.gpsimd.partition_broadcast(cb[:], cs[:])
                    og = og_pool.tile([128, NB], BF, tag=f"og{h}")
                    nc.vector.tensor_mul(og[:], po[:], cb[:])
                    ogs.append(og)

            # ---- output projection: out[jn block] = sum_h og_h^T @ Wo_h ----
            with (
                tc.tile_pool(name="ps_out", bufs=2, space="PSUM") as ps_out,
                tc.tile_pool(name="o_sb", bufs=2) as o_pool,
            ):
                for jn in range(4):
                    o_sb = o_pool.tile([128, D], F32, tag="o")
                    for cc in range(4):
                        pO = ps_out.tile([128, NB], F32, tag="pO")
                        for h in range(H):
                            nc.tensor.matmul(
                                pO[:],
                                ogs[h][:, jn * 128:(jn + 1) * 128],
                                wo_sb[:, h * D + cc * NB:h * D + (cc + 1) * NB],
                                start=(h == 0),
                                stop=(h == H - 1),
                            )
                        nc.scalar.activation(o_sb[:, cc * NB:(cc + 1) * NB], pO[:], ACT.Copy)
                    nc.sync.dma_start(out[jn * 128:(jn + 1) * 128, :], o_sb[:])

    nc.finalize()
    return nc


def _get_nc():
    if "nc" not in _CACHE:
        _CACHE["nc"] = _build()
    return _CACHE["nc"]


def _prep_in_maps(x, rotary_pos_emb, Wq, Wk, Wv, Wg, bg, Wo):
    cosT = np.cos(rotary_pos_emb.astype(np.float64)).T.astype(np.float32)  # (128, 2048)
    sinT = np.sin(rotary_pos_emb.astype(np.float64)).T.astype(np.float32)
    sgn = np.concatenate([-np.ones(64), np.ones(64)]).astype(np.float32)[:, None]
    sinTs = sinT * sgn

    wq_b = np.ascontiguousarray(Wq).astype(bf16)
    wk_b = np.ascontiguousarray(Wk).astype(bf16)
    wv_b = np.ascontiguousarray(Wv).astype(bf16)
    wg_b = np.ascontiguousarray(Wg).astype(bf16)
    wo_b = np.ascontiguousarray(Wo).astype(bf16)
    bg_2 = np.ascontiguousarray(bg.reshape(H, 1)).astype(np.float32)

    in_maps = []
    for c in range(NCORES):
        b, r = divmod(c, 4)
        sl = slice(r * NB, (r + 1) * NB)
        in_maps.append({
            "xt": np.ascontiguousarray(x[b, sl, :].T).astype(bf16),
            "cos": np.ascontiguousarray(cosT[:, sl]).astype(bf16),
            "sins": np.ascontiguousarray(sinTs[:, sl]).astype(bf16),
            "wq": wq_b, "wk": wk_b, "wv": wv_b, "wg": wg_b,
            "bg": bg_2, "wo": wo_b,
        })
    return in_maps


def run(x, rotary_pos_emb, Wq, Wk, Wv, Wg, bg, Wo, trace=False):
    from concourse.bass_utils import run_bass_kernel_spmd

    nc = _get_nc()
    in_maps = _prep_in_maps(x, rotary_pos_emb, Wq, Wk, Wv, Wg, bg, Wo)
    kwargs = {}
    if trace:
        kwargs = dict(trace=True, trace_cores=list(range(NCORES)))
    res = run_bass_kernel_spmd(nc, in_maps, core_ids=list(range(NCORES)), **kwargs)
    full = np.empty((B, N, D), dtype=np.float32)
    for c in range(NCORES):
        b, r = divmod(c, 4)
        full[b, r * NB:(r + 1) * NB, :] = res.results[c]["out"]
    return full, res


def kernel(x, rotary_pos_emb, Wq, Wk, Wv, Wg, bg, Wo):
    full, _ = run(x, rotary_pos_emb, Wq, Wk, Wv, Wg, bg, Wo)
    return full


# revision 9
# speedup vs baseline: 1.0022x; 1.0022x over previous
"""Distributed Trainium2 (8 NeuronCore) kernel for nn_Attention_54382875902030.

Reference computation (B=2, N=2048, D=2048, H=16, DH=128):
    q,k,v = split_heads(x @ Wq/Wk/Wv);  RoPE(q), RoPE(k)
    out   = softmax(q k^T / sqrt(DH)) v
    out  *= sigmoid(x @ Wg + bg)  (per-head gate)
    return merge_heads(out) @ Wo

Sharding: 8 cores = 2 batch groups x 4 sequence blocks (512 query rows each).
Each core projects q/k/v/gate for its own 512 rows (all 16 heads), then
AllGathers k^T and v within its batch group (replica groups [[0..3],[4..7]]),
runs full non-causal attention for its 512 query rows over all 2048 keys,
and computes its block of the output projection locally (full Wo). The
output is a disjoint row-slice per core -> host-side gather is a pure
concatenation (no reduction).

The AllGathers are split per head-half (k0, v0, k1, v1) and interleaved
with the projection phases so the collectives overlap projection compute
and the second half's collective overlaps the first half's attention.

All weights/activations are host-repacked into the exact SBUF tile layouts
so every DMA is a contiguous [128, F] block load.

All matmuls run in bf16 (fp32 PSUM accumulation). Attention is computed in
the transposed layout S^T[k, q] = k . q^T so that:
  - exp(SCALE * S^T) needs no per-row bias (inputs are ~N(0,1) -> scores
    bounded well inside fp32/bf16 exp range, max-subtraction skipped),
  - softmax denominators come from a ones-vector matmul (partition-axis sum),
  - P^T feeds the PV matmul directly (no transposes anywhere in the kernel).
"""

import numpy as np
import ml_dtypes

bf16 = ml_dtypes.bfloat16

B, N, D = 2, 2048, 2048
H, DH = 16, 128
NB = N // 4          # 512 local query rows per core
DC = D // 128        # 16 contraction chunks
KC = N // 128        # 16 key chunks
HH = H // 2          # heads per AG half
SCALE = DH ** -0.5
NCORES = 8
GROUPS = [[0, 1, 2, 3], [4, 5, 6, 7]]

_CACHE = {}


def _build():
    import concourse.bacc as bacc
    import concourse.tile as tile
    import concourse.mybir as mybir

    F32 = mybir.dt.float32
    BF = mybir.dt.bfloat16
    ACT = mybir.ActivationFunctionType

    nc = bacc.Bacc(None, target_bir_lowering=False, num_devices=NCORES)

    # ---- parameters, all pre-packed host-side for contiguous DMA ----
    # xt: [128, (dchunk, n)]  = x[b, rows].T tiled
    xt = nc.declare_dram_parameter("xt", [128, DC * NB], BF, isOutput=False)
    cos = nc.declare_dram_parameter("cos", [DH, NB], BF, isOutput=False)
    sins = nc.declare_dram_parameter("sins", [DH, NB], BF, isOutput=False)
    # wq/wk: rows h*128+p, cols (dchunk, dh-col)
    wq = nc.declare_dram_parameter("wq", [H * 128, DC * DH], BF, isOutput=False)
    wk = nc.declare_dram_parameter("wk", [H * 128, DC * DH], BF, isOutput=False)
    # wv: rows cc*128+p, cols (dchunk, n-col)
    wv = nc.declare_dram_parameter("wv", [4 * 128, DC * NB], BF, isOutput=False)
    # wg: [128, (dchunk, head)]
    wg = nc.declare_dram_parameter("wg", [128, DC * H], BF, isOutput=False)
    bg = nc.declare_dram_parameter("bg", [H, 1], F32, isOutput=False)
    # wo: [128, (head, cols)]
    wo = nc.declare_dram_parameter("wo", [128, H * D], BF, isOutput=False)
    out = nc.declare_dram_parameter("out", [NB, D], F32, isOutput=True)

    ones_c = nc.inline_tensor(np.ones((128, 1), bf16), name="ones_c")

    with tile.TileContext(nc) as tc:
        with (
            tc.tile_pool(name="dram", bufs=1, space="DRAM") as dram,
            tc.tile_pool(name="persist", bufs=1) as persist,
            tc.tile_pool(name="qt_pool", bufs=1) as qt_pool,
            tc.tile_pool(name="og_pool", bufs=1) as og_pool,
        ):
            # collective bounce buffers (one pair per head-half)
            k_in = [dram.tile([HH * DH, NB], BF, name=f"k_in{i}") for i in range(2)]
            k_out = [dram.tile([4 * HH * DH, NB], BF, name=f"k_out{i}") for i in range(2)]
            v_in = [dram.tile([NB, HH * DH], BF, name=f"v_in{i}") for i in range(2)]
            v_out = [dram.tile([4 * NB, HH * DH], BF, name=f"v_out{i}") for i in range(2)]

            # ---- constants & activations resident in SBUF ----
            xt_sb = persist.tile([128, DC * NB], BF)
            nc.sync.dma_start(xt_sb[:], xt.ap())
            cos_sb = persist.tile([128, NB], BF)
            sins_sb = persist.tile([128, NB], BF)
            nc.sync.dma_start(cos_sb[:], cos.ap())
            nc.sync.dma_start(sins_sb[:], sins.ap())
            bg_sb = persist.tile([H, 1], F32)
            nc.sync.dma_start(bg_sb[:], bg.ap())
            ones_sb = persist.tile([128, 1], BF)
            nc.sync.dma_start(ones_sb[:], ones_c.ap())
            gate_sb = persist.tile([H, NB], BF)
            wo_sb = persist.tile([128, H * D], BF)
            v_loc = persist.tile([128, 4 * 4 * NB], BF)   # free=(jn, cc, n)

            def qk_proj_head(w, h, wpool, psum, evac, dst_sb):
                """project head h of w -> RoPE -> dst_sb [128(dh), NB] bf16."""
                w_sb = wpool.tile([128, DC * DH], BF, tag="wqk")
                nc.sync.dma_start(w_sb[:], w.ap()[h * 128:(h + 1) * 128, :])
                pk = psum.tile([128, NB], F32, tag="pacc")
                for dc in range(DC):
                    nc.tensor.matmul(
                        pk[:],
                        w_sb[:, dc * DH:(dc + 1) * DH],
                        xt_sb[:, dc * NB:(dc + 1) * NB],
                        start=(dc == 0),
                        stop=(dc == DC - 1),
                    )
                # RoPE: dst = t*cos + rot(t)*sins   (sins rows 0:64 pre-negated)
                tf = evac.tile([128, NB], BF, tag="tf")
                nc.scalar.activation(tf[:], pk[:], ACT.Copy)
                t1 = evac.tile([128, NB], BF, tag="t1")
                nc.vector.tensor_mul(t1[:], tf[:], cos_sb[:])
                # rotate-half via address-based SBUF->SBUF DMA (engines can't
                # mix SBUF base partitions within one tensor_tensor)
                rot = evac.tile([128, NB], BF, tag="rot")
                nc.sync.dma_start(rot[0:64, :], tf[64:128, :])
                nc.sync.dma_start(rot[64:128, :], tf[0:64, :])
                t2 = evac.tile([128, NB], BF, tag="t2")
                nc.vector.tensor_mul(t2[:], rot[:], sins_sb[:])
                nc.vector.tensor_add(dst_sb[:], t1[:], t2[:])

            def v_proj_chunk(cc, wpool, psum):
                """v columns cc*512:(cc+1)*512 into v_loc (natural layout)."""
                wv_sb = wpool.tile([128, DC * NB], BF, tag="wv")
                nc.sync.dma_start(wv_sb[:], wv.ap()[cc * 128:(cc + 1) * 128, :])
                for jn in range(4):
                    pv = psum.tile([128, NB], F32, tag="pacc")
                    for dc in range(DC):
                        nc.tensor.matmul(
                            pv[:],
                            xt_sb[:, dc * NB + jn * 128:dc * NB + (jn + 1) * 128],
                            wv_sb[:, dc * NB:(dc + 1) * NB],
                            start=(dc == 0),
                            stop=(dc == DC - 1),
                        )
                    nc.scalar.activation(
                        v_loc[:, (jn * 4 + cc) * NB:(jn * 4 + cc + 1) * NB],
                        pv[:],
                        ACT.Copy,
                    )

            def ag(i_in, i_out):
                nc.gpsimd.collective_compute(
                    "AllGather",
                    mybir.AluOpType.bypass,
                    replica_groups=GROUPS,
                    ins=[i_in.opt()],
                    outs=[i_out.opt()],
                )

            with (
                tc.tile_pool(name="wpool", bufs=2) as wpool,
                tc.tile_pool(name="psum_proj", bufs=2, space="PSUM") as psum,
                tc.tile_pool(name="evac", bufs=2) as evac,
                tc.tile_pool(name="kt_pool", bufs=3) as kt_pool,
            ):
                for half in range(2):
                    # k projection for this head-half -> AG
                    for hh in range(HH):
                        h = half * HH + hh
                        kt = kt_pool.tile([128, NB], BF, tag="kt")
                        qk_proj_head(wk, h, wpool, psum, evac, kt)
                        nc.sync.dma_start(k_in[half][hh * DH:(hh + 1) * DH, :], kt[:])
                    ag(k_in[half], k_out[half])
                    # v projection for this head-half (cols half*1024 ..) -> AG
                    for cc in (2 * half, 2 * half + 1):
                        v_proj_chunk(cc, wpool, psum)
                    nc.sync.dma_start(
                        v_in[half].rearrange("(jn p) (cc n) -> p jn cc n", p=128, cc=2),
                        v_loc[:].rearrange("p (jn cc n) -> p jn cc n", jn=4, cc=4)[
                            :, :, 2 * half:2 * half + 2, :
                        ],
                    )
                    ag(v_in[half], v_out[half])

                # ---- q projection + RoPE (kept in SBUF) ----
                qts = []
                for h in range(H):
                    qt = qt_pool.tile([128, NB], BF, tag=f"qt{h}")
                    qk_proj_head(wq, h, wpool, psum, evac, qt)
                    qts.append(qt)

                # ---- gate: sigmoid(Wg^T x^T + bg) -> [H, NB] ----
                wg_sb = persist.tile([128, DC * H], BF)
                nc.sync.dma_start(wg_sb[:], wg.ap())
                pg = psum.tile([H, NB], F32, tag="pg")
                for dc in range(DC):
                    nc.tensor.matmul(
                        pg[:],
                        wg_sb[:, dc * H:(dc + 1) * H],
                        xt_sb[:, dc * NB:(dc + 1) * NB],
                        start=(dc == 0),
                        stop=(dc == DC - 1),
                    )
                nc.scalar.activation(gate_sb[:], pg[:], ACT.Sigmoid, bias=bg_sb[:])

            # ---- attention: 2 halves x 8 heads x [2048 k, 512 q] ----
            ogs = []
            with (
                tc.tile_pool(name="v_all_pool", bufs=1) as v_all_pool,
                tc.tile_pool(name="k_h_pool", bufs=3) as k_h_pool,
                tc.tile_pool(name="ps_s", bufs=2, space="PSUM") as ps_s,
                tc.tile_pool(name="ps_o", bufs=2, space="PSUM") as ps_o,
                tc.tile_pool(name="ps_l", bufs=2, space="PSUM") as ps_l,
                tc.tile_pool(name="p_pool", bufs=3) as p_pool,
                tc.tile_pool(name="smalls", bufs=2) as smalls,
            ):
                for half in range(2):
                    # whole half of gathered v -> SBUF in one contiguous DMA
                    v_all = v_all_pool.tile([128, KC * HH * DH], BF, tag="v_all")
                    nc.sync.dma_start(
                        v_all[:].rearrange("p (ck m) -> p ck m", ck=KC),
                        v_out[half].rearrange("(ck p) m -> p ck m", p=128),
                    )
                    k_out_r = k_out[half].rearrange("(r h p) n -> p h r n", h=HH, p=128)
                    for hh in range(HH):
                        h = half * HH + hh
                        kt_h = k_h_pool.tile([128, N], BF, tag="kt_h")
                        nc.sync.dma_start(
                            kt_h[:].rearrange("p (r n) -> p r n", r=4),
                            k_out_r[:, hh],
                        )
                        po = ps_o.tile([128, NB], F32, tag="po")
                        pl = ps_l.tile([1, NB], F32, tag="pl")
                        for pr in range(KC // 2):
                            ps = ps_s.tile([128, 2 * NB], F32, tag="ps")
                            for sub in range(2):
                                ik = 2 * pr + sub
                                nc.tensor.matmul(
                                    ps[:, sub * NB:(sub + 1) * NB],
                                    kt_h[:, ik * 128:(ik + 1) * 128],
                                    qts[h][:],
                                    start=True,
                                    stop=True,
                                )
                            pexp = p_pool.tile([128, 2 * NB], BF, tag="pexp")
                            nc.scalar.activation(pexp[:], ps[:], ACT.Exp, scale=SCALE)
                            for sub in range(2):
                                ik = 2 * pr + sub
                                nc.tensor.matmul(
                                    po[:],
                                    v_all[:, ik * HH * DH + hh * DH:
                                          ik * HH * DH + (hh + 1) * DH],
                                    pexp[:, sub * NB:(sub + 1) * NB],
                                    start=(ik == 0),
                                    stop=(ik == KC - 1),
                                )
                                nc.tensor.matmul(
                                    pl[:],
                                    ones_sb[:],
                                    pexp[:, sub * NB:(sub + 1) * NB],
                                    start=(ik == 0),
                                    stop=(ik == KC - 1),
                                )
                        # c = sigmoid-gate / softmax-sum, broadcast over partitions
                        lr = smalls.tile([1, NB], F32, tag="lr")
                        nc.vector.reciprocal(lr[:], pl[:])
                        gh = smalls.tile([1, NB], BF, tag="gh")
                        nc.sync.dma_start(gh[:], gate_sb[h:h + 1, :])
                        cs = smalls.tile([1, NB], BF, tag="cs")
                        nc.vector.tensor_mul(cs[:], lr[:], gh[:])
                        cb = smalls.tile([128, NB], BF, tag="cb")
                        nc.gpsimd.partition_broadcast(cb[:], cs[:])
                        og = og_pool.tile([128, NB], BF, tag=f"og{h}")
                        nc.vector.tensor_mul(og[:], po[:], cb[:])
                        ogs.append(og)

            # Wo load (off critical path; needed from the first out-proj matmul)
            nc.sync.dma_start(wo_sb[:], wo.ap())

            # ---- output projection: out[jn block] = sum_h og_h^T @ Wo_h ----
            with (
                tc.tile_pool(name="ps_out", bufs=2, space="PSUM") as ps_out,
                tc.tile_pool(name="o_sb", bufs=2) as o_pool,
            ):
                for jn in range(4):
                    o_sb = o_pool.tile([128, D], F32, tag="o")
                    for cc in range(4):
                        pO = ps_out.tile([128, NB], F32, tag="pO")
                        for h in range(H):
                            nc.tensor.matmul(
                                pO[:],
                                ogs[h][:, jn * 128:(jn + 1) * 128],
                                wo_sb[:, h * D + cc * NB:h * D + (cc + 1) * NB],
                                start=(h == 0),
                                stop=(h == H - 1),
                            )
                        nc.scalar.activation(o_sb[:, cc * NB:(cc + 1) * NB], pO[:], ACT.Copy)
                    nc.sync.dma_start(out[jn * 128:(jn + 1) * 128, :], o_sb[:])

    nc.finalize()
    return nc


def _get_nc():
    if "nc" not in _CACHE:
        _CACHE["nc"] = _build()
    return _CACHE["nc"]


def _prep_in_maps(x, rotary_pos_emb, Wq, Wk, Wv, Wg, bg, Wo):
    cosT = np.cos(rotary_pos_emb.astype(np.float64)).T.astype(np.float32)  # (128, 2048)
    sinT = np.sin(rotary_pos_emb.astype(np.float64)).T.astype(np.float32)
    sgn = np.concatenate([-np.ones(64), np.ones(64)]).astype(np.float32)[:, None]
    sinTs = sinT * sgn

    # pack weights into the exact SBUF tile layouts (see _build)
    wq_p = np.ascontiguousarray(
        np.asarray(Wq).reshape(DC, 128, H, DH).transpose(2, 1, 0, 3).reshape(H * 128, DC * DH)
    ).astype(bf16)
    wk_p = np.ascontiguousarray(
        np.asarray(Wk).reshape(DC, 128, H, DH).transpose(2, 1, 0, 3).reshape(H * 128, DC * DH)
    ).astype(bf16)
    wv_p = np.ascontiguousarray(
        np.asarray(Wv).reshape(DC, 128, 4, NB).transpose(2, 1, 0, 3).reshape(4 * 128, DC * NB)
    ).astype(bf16)
    wg_p = np.ascontiguousarray(
        np.asarray(Wg).reshape(DC, 128, H).transpose(1, 0, 2).reshape(128, DC * H)
    ).astype(bf16)
    wo_p = np.ascontiguousarray(
        np.asarray(Wo).reshape(H, 128, D).transpose(1, 0, 2).reshape(128, H * D)
    ).astype(bf16)
    bg_2 = np.ascontiguousarray(np.asarray(bg).reshape(H, 1)).astype(np.float32)

    in_maps = []
    for c in range(NCORES):
        b, r = divmod(c, 4)
        sl = slice(r * NB, (r + 1) * NB)
        xt_p = np.ascontiguousarray(
            np.asarray(x[b, sl, :]).reshape(NB, DC, 128).transpose(2, 1, 0).reshape(128, DC * NB)
        ).astype(bf16)
        in_maps.append({
            "xt": xt_p,
            "cos": np.ascontiguousarray(cosT[:, sl]).astype(bf16),
            "sins": np.ascontiguousarray(sinTs[:, sl]).astype(bf16),
            "wq": wq_p, "wk": wk_p, "wv": wv_p, "wg": wg_p,
            "bg": bg_2, "wo": wo_p,
        })
    return in_maps


def run(x, rotary_pos_emb, Wq, Wk, Wv, Wg, bg, Wo, trace=False):
    from concourse.bass_utils import run_bass_kernel_spmd

    nc = _get_nc()
    in_maps = _prep_in_maps(x, rotary_pos_emb, Wq, Wk, Wv, Wg, bg, Wo)
    kwargs = {}
    if trace:
        kwargs = dict(trace=True, trace_cores=list(range(NCORES)))
    res = run_bass_kernel_spmd(nc, in_maps, core_ids=list(range(NCORES)), **kwargs)
    full = np.empty((B, N, D), dtype=np.float32)
    for c in range(NCORES):
        b, r = divmod(c, 4)
        full[b, r * NB:(r + 1) * NB, :] = res.results[c]["out"]
    return full, res


def kernel(x, rotary_pos_emb, Wq, Wk, Wv, Wg, bg, Wo):
    full, _ = run(x, rotary_pos_emb, Wq, Wk, Wv, Wg, bg, Wo)
    return full


# revision 11
# speedup vs baseline: 1.0948x; 1.0924x over previous
"""Distributed Trainium2 (8 NeuronCore) kernel for nn_Attention_54382875902030.

Reference computation (B=2, N=2048, D=2048, H=16, DH=128):
    q,k,v = split_heads(x @ Wq/Wk/Wv);  RoPE(q), RoPE(k)
    out   = softmax(q k^T / sqrt(DH)) v
    out  *= sigmoid(x @ Wg + bg)  (per-head gate)
    return merge_heads(out) @ Wo

Sharding: 8 cores = 2 batch groups x 4 sequence blocks (512 query rows each).
Each core projects q/k/v/gate for its own 512 rows (all 16 heads), then
AllGathers k^T and v within its batch group (replica groups [[0..3],[4..7]]),
runs full non-causal attention for its 512 query rows over all 2048 keys,
and computes its block of the output projection locally (full Wo). The
output is a disjoint row-slice per core -> host-side gather is a pure
concatenation (no reduction).

The AllGathers are split per head-half (k0, v0, k1, v1) and interleaved
with the projection phases so the collectives overlap projection compute
and the second half's collective overlaps the first half's attention.

All weights/activations are host-repacked into the exact SBUF tile layouts
so every DMA is a contiguous [128, F] block load.

All matmuls run in bf16 (fp32 PSUM accumulation). Attention is computed in
the transposed layout S^T[k, q] = k . q^T so that:
  - exp(SCALE * S^T) needs no per-row bias (inputs are ~N(0,1) -> scores
    bounded well inside fp32/bf16 exp range, max-subtraction skipped),
  - softmax denominators come from a ones-vector matmul (partition-axis sum),
  - P^T feeds the PV matmul directly (no transposes anywhere in the kernel).
"""

import numpy as np
import ml_dtypes

bf16 = ml_dtypes.bfloat16

B, N, D = 2, 2048, 2048
H, DH = 16, 128
NB = N // 4          # 512 local query rows per core
DC = D // 128        # 16 contraction chunks
KC = N // 128        # 16 key chunks
HH = H // 2          # heads per AG half
SCALE = DH ** -0.5
NCORES = 8
GROUPS = [[0, 1, 2, 3], [4, 5, 6, 7]]

_CACHE = {}


def _patch_compiler_flags():
    import concourse.bass_utils as _bu
    if getattr(_bu, "_ldw_patched", False):
        return
    _orig = _bu.run_command

    def _run(cmd, *a, **k):
        if isinstance(cmd, list):
            cmd = [c
                   for c in cmd]
        return _orig(cmd, *a, **k)

    _bu.run_command = _run
    _bu._ldw_patched = True


def _build():
    _patch_compiler_flags()
    import concourse.bacc as bacc
    import concourse.tile as tile
    import concourse.mybir as mybir

    F32 = mybir.dt.float32
    BF = mybir.dt.bfloat16
    ACT = mybir.ActivationFunctionType

    nc = bacc.Bacc(None, target_bir_lowering=False, num_devices=NCORES)

    # ---- parameters, all pre-packed host-side for contiguous DMA ----
    # xt: [128, (dchunk, n)]  = x[b, rows].T tiled
    xt = nc.declare_dram_parameter("xt", [128, DC * NB], BF, isOutput=False)
    cos = nc.declare_dram_parameter("cos", [DH, NB], BF, isOutput=False)
    sins = nc.declare_dram_parameter("sins", [DH, NB], BF, isOutput=False)
    # wq/wk: rows h*128+p, cols (dchunk, dh-col)
    wq = nc.declare_dram_parameter("wq", [H * 128, DC * DH], BF, isOutput=False)
    wk = nc.declare_dram_parameter("wk", [H * 128, DC * DH], BF, isOutput=False)
    # wv: rows cc*128+p, cols (dchunk, n-col)
    wv = nc.declare_dram_parameter("wv", [4 * 128, DC * NB], BF, isOutput=False)
    # wg: [128, (dchunk, head)]
    wg = nc.declare_dram_parameter("wg", [128, DC * H], BF, isOutput=False)
    bg = nc.declare_dram_parameter("bg", [H, 1], F32, isOutput=False)
    # wo: [128, (head, cols)]
    wo = nc.declare_dram_parameter("wo", [128, H * D], BF, isOutput=False)
    out = nc.declare_dram_parameter("out", [NB, D], F32, isOutput=True)

    ones_c = nc.inline_tensor(np.ones((128, 1), bf16), name="ones_c")

    with tile.TileContext(nc) as tc:
        with (
            tc.tile_pool(name="dram", bufs=1, space="DRAM") as dram,
            tc.tile_pool(name="persist", bufs=1) as persist,
            tc.tile_pool(name="qt_pool", bufs=1) as qt_pool,
            tc.tile_pool(name="og_pool", bufs=1) as og_pool,
        ):
            # collective bounce buffers (one pair per head-half)
            k_in = [dram.tile([HH * DH, NB], BF, name=f"k_in{i}") for i in range(2)]
            k_out = [dram.tile([4 * HH * DH, NB], BF, name=f"k_out{i}") for i in range(2)]
            v_in = [dram.tile([NB, HH * DH], BF, name=f"v_in{i}") for i in range(2)]
            v_out = [dram.tile([4 * NB, HH * DH], BF, name=f"v_out{i}") for i in range(2)]

            # ---- constants & activations resident in SBUF ----
            xt_sb = persist.tile([128, DC * NB], BF)
            nc.sync.dma_start(xt_sb[:], xt.ap())
            cos_sb = persist.tile([128, NB], BF)
            sins_sb = persist.tile([128, NB], BF)
            nc.sync.dma_start(cos_sb[:], cos.ap())
            nc.sync.dma_start(sins_sb[:], sins.ap())
            bg_sb = persist.tile([H, 1], F32)
            nc.sync.dma_start(bg_sb[:], bg.ap())
            ones_sb = persist.tile([128, 1], BF)
            nc.sync.dma_start(ones_sb[:], ones_c.ap())
            gate_sb = persist.tile([H, NB], BF)
            wo_sb = persist.tile([128, H * D], BF)
            v_loc = persist.tile([128, 4 * 4 * NB], BF)   # free=(jn, cc, n)

            def qk_proj_head(w, h, wpool, psum, evac, dst_sb):
                """project head h of w -> RoPE -> dst_sb [128(dh), NB] bf16."""
                w_sb = wpool.tile([128, DC * DH], BF, tag="wqk")
                nc.sync.dma_start(w_sb[:], w.ap()[h * 128:(h + 1) * 128, :])
                pk = psum.tile([128, NB], F32, tag="pacc")
                for dc in range(DC):
                    nc.tensor.matmul(
                        pk[:],
                        w_sb[:, dc * DH:(dc + 1) * DH],
                        xt_sb[:, dc * NB:(dc + 1) * NB],
                        start=(dc == 0),
                        stop=(dc == DC - 1),
                    )
                # RoPE: dst = t*cos + rot(t)*sins   (sins rows 0:64 pre-negated)
                tf = evac.tile([128, NB], BF, tag="tf")
                nc.scalar.activation(tf[:], pk[:], ACT.Copy)
                t1 = evac.tile([128, NB], BF, tag="t1")
                nc.vector.tensor_mul(t1[:], tf[:], cos_sb[:])
                # rotate-half via address-based SBUF->SBUF DMA (engines can't
                # mix SBUF base partitions within one tensor_tensor)
                rot = evac.tile([128, NB], BF, tag="rot")
                nc.sync.dma_start(rot[0:64, :], tf[64:128, :])
                nc.sync.dma_start(rot[64:128, :], tf[0:64, :])
                t2 = evac.tile([128, NB], BF, tag="t2")
                nc.vector.tensor_mul(t2[:], rot[:], sins_sb[:])
                nc.vector.tensor_add(dst_sb[:], t1[:], t2[:])

            def v_proj_chunk(cc, wpool, psum):
                """v columns cc*512:(cc+1)*512 into v_loc (natural layout)."""
                wv_sb = wpool.tile([128, DC * NB], BF, tag="wv", bufs=2)
                nc.sync.dma_start(wv_sb[:], wv.ap()[cc * 128:(cc + 1) * 128, :])
                for jn in range(4):
                    pv = psum.tile([128, NB], F32, tag="pacc")
                    for dc in range(DC):
                        nc.tensor.matmul(
                            pv[:],
                            xt_sb[:, dc * NB + jn * 128:dc * NB + (jn + 1) * 128],
                            wv_sb[:, dc * NB:(dc + 1) * NB],
                            start=(dc == 0),
                            stop=(dc == DC - 1),
                        )
                    nc.scalar.activation(
                        v_loc[:, (jn * 4 + cc) * NB:(jn * 4 + cc + 1) * NB],
                        pv[:],
                        ACT.Copy,
                    )

            def ag(i_in, i_out):
                nc.gpsimd.collective_compute(
                    "AllGather",
                    mybir.AluOpType.bypass,
                    replica_groups=GROUPS,
                    ins=[i_in.opt()],
                    outs=[i_out.opt()],
                )

            with (
                tc.tile_pool(name="wpool", bufs=6) as wpool,
                tc.tile_pool(name="psum_proj", bufs=2, space="PSUM") as psum,
                tc.tile_pool(name="evac", bufs=2) as evac,
                tc.tile_pool(name="kt_pool", bufs=3) as kt_pool,
            ):
                for half in range(2):
                    # k projection for this head-half -> AG
                    for hh in range(HH):
                        h = half * HH + hh
                        kt = kt_pool.tile([128, NB], BF, tag="kt")
                        qk_proj_head(wk, h, wpool, psum, evac, kt)
                        nc.sync.dma_start(k_in[half][hh * DH:(hh + 1) * DH, :], kt[:])
                    ag(k_in[half], k_out[half])
                    # v projection for this head-half (cols half*1024 ..) -> AG
                    for cc in (2 * half, 2 * half + 1):
                        v_proj_chunk(cc, wpool, psum)
                    nc.sync.dma_start(
                        v_in[half].rearrange("(jn p) (cc n) -> p jn cc n", p=128, cc=2),
                        v_loc[:].rearrange("p (jn cc n) -> p jn cc n", jn=4, cc=4)[
                            :, :, 2 * half:2 * half + 2, :
                        ],
                    )
                    ag(v_in[half], v_out[half])

                # ---- q projection + RoPE (kept in SBUF) ----
                qts = []
                for h in range(H):
                    qt = qt_pool.tile([128, NB], BF, tag=f"qt{h}")
                    qk_proj_head(wq, h, wpool, psum, evac, qt)
                    qts.append(qt)

                # ---- gate: sigmoid(Wg^T x^T + bg) -> [H, NB] ----
                wg_sb = persist.tile([128, DC * H], BF)
                nc.sync.dma_start(wg_sb[:], wg.ap())
                pg = psum.tile([H, NB], F32, tag="pg")
                for dc in range(DC):
                    nc.tensor.matmul(
                        pg[:],
                        wg_sb[:, dc * H:(dc + 1) * H],
                        xt_sb[:, dc * NB:(dc + 1) * NB],
                        start=(dc == 0),
                        stop=(dc == DC - 1),
                    )
                nc.scalar.activation(gate_sb[:], pg[:], ACT.Sigmoid, bias=bg_sb[:])

            # ---- attention: 2 halves x 8 heads x [2048 k, 512 q] ----
            ogs = []
            with (
                tc.tile_pool(name="v_all_pool", bufs=1) as v_all_pool,
                tc.tile_pool(name="k_h_pool", bufs=3) as k_h_pool,
                tc.tile_pool(name="ps_s", bufs=2, space="PSUM") as ps_s,
                tc.tile_pool(name="ps_o", bufs=2, space="PSUM") as ps_o,
                tc.tile_pool(name="ps_l", bufs=2, space="PSUM") as ps_l,
                tc.tile_pool(name="p_pool", bufs=3) as p_pool,
                tc.tile_pool(name="smalls", bufs=2) as smalls,
            ):
                for half in range(2):
                    # whole half of gathered v -> SBUF in one contiguous DMA
                    v_all = v_all_pool.tile([128, KC * HH * DH], BF, tag="v_all")
                    nc.sync.dma_start(
                        v_all[:].rearrange("p (ck m) -> p ck m", ck=KC),
                        v_out[half].rearrange("(ck p) m -> p ck m", p=128),
                    )
                    k_out_r = k_out[half].rearrange("(r h p) n -> p h r n", h=HH, p=128)
                    for hh in range(HH):
                        h = half * HH + hh
                        kt_h = k_h_pool.tile([128, N], BF, tag="kt_h")
                        nc.sync.dma_start(
                            kt_h[:].rearrange("p (r n) -> p r n", r=4),
                            k_out_r[:, hh],
                        )
                        po = ps_o.tile([128, NB], F32, tag="po")
                        pl = ps_l.tile([1, NB], F32, tag="pl")
                        for pr in range(KC // 2):
                            ps = ps_s.tile([128, 2 * NB], F32, tag="ps")
                            for sub in range(2):
                                ik = 2 * pr + sub
                                nc.tensor.matmul(
                                    ps[:, sub * NB:(sub + 1) * NB],
                                    kt_h[:, ik * 128:(ik + 1) * 128],
                                    qts[h][:],
                                    start=True,
                                    stop=True,
                                )
                            pexp = p_pool.tile([128, 2 * NB], BF, tag="pexp")
                            nc.scalar.activation(pexp[:], ps[:], ACT.Exp, scale=SCALE)
                            for sub in range(2):
                                ik = 2 * pr + sub
                                nc.tensor.matmul(
                                    po[:],
                                    v_all[:, ik * HH * DH + hh * DH:
                                          ik * HH * DH + (hh + 1) * DH],
                                    pexp[:, sub * NB:(sub + 1) * NB],
                                    start=(ik == 0),
                                    stop=(ik == KC - 1),
                                )
                                nc.tensor.matmul(
                                    pl[:],
                                    ones_sb[:],
                                    pexp[:, sub * NB:(sub + 1) * NB],
                                    start=(ik == 0),
                                    stop=(ik == KC - 1),
                                )
                        # c = sigmoid-gate / softmax-sum, broadcast over partitions
                        lr = smalls.tile([1, NB], F32, tag="lr")
                        nc.vector.reciprocal_approx_fast(lr[:], pl[:])
                        gh = smalls.tile([1, NB], BF, tag="gh")
                        nc.sync.dma_start(gh[:], gate_sb[h:h + 1, :])
                        cs = smalls.tile([1, NB], BF, tag="cs")
                        nc.vector.tensor_mul(cs[:], lr[:], gh[:])
                        cb = smalls.tile([128, NB], BF, tag="cb")
                        nc.gpsimd.partition_broadcast(cb[:], cs[:])
                        og = og_pool.tile([128, NB], BF, tag=f"og{h}")
                        nc.vector.tensor_mul(og[:], po[:], cb[:])
                        ogs.append(og)

            # Wo load (off critical path; needed from the first out-proj matmul)
            nc.sync.dma_start(wo_sb[:], wo.ap())

            # ---- output projection: out[jn block] = sum_h og_h^T @ Wo_h ----
            with (
                tc.tile_pool(name="ps_out", bufs=2, space="PSUM") as ps_out,
                tc.tile_pool(name="o_sb", bufs=2) as o_pool,
            ):
                for jn in range(4):
                    o_sb = o_pool.tile([128, D], F32, tag="o")
                    for cc in range(4):
                        pO = ps_out.tile([128, NB], F32, tag="pO")
                        for h in range(H):
                            nc.tensor.matmul(
                                pO[:],
                                ogs[h][:, jn * 128:(jn + 1) * 128],
                                wo_sb[:, h * D + cc * NB:h * D + (cc + 1) * NB],
                                start=(h == 0),
                                stop=(h == H - 1),
                            )
                        nc.scalar.activation(o_sb[:, cc * NB:(cc + 1) * NB], pO[:], ACT.Copy)
                    nc.sync.dma_start(out[jn * 128:(jn + 1) * 128, :], o_sb[:])

    nc.finalize()
    return nc


def _get_nc():
    if "nc" not in _CACHE:
        _CACHE["nc"] = _build()
    return _CACHE["nc"]


def _prep_in_maps(x, rotary_pos_emb, Wq, Wk, Wv, Wg, bg, Wo):
    cosT = np.cos(rotary_pos_emb.astype(np.float64)).T.astype(np.float32)  # (128, 2048)
    sinT = np.sin(rotary_pos_emb.astype(np.float64)).T.astype(np.float32)
    sgn = np.concatenate([-np.ones(64), np.ones(64)]).astype(np.float32)[:, None]
    sinTs = sinT * sgn

    # pack weights into the exact SBUF tile layouts (see _build)
    wq_p = np.ascontiguousarray(
        np.asarray(Wq).reshape(DC, 128, H, DH).transpose(2, 1, 0, 3).reshape(H * 128, DC * DH)
    ).astype(bf16)
    wk_p = np.ascontiguousarray(
        np.asarray(Wk).reshape(DC, 128, H, DH).transpose(2, 1, 0, 3).reshape(H * 128, DC * DH)
    ).astype(bf16)
    wv_p = np.ascontiguousarray(
        np.asarray(Wv).reshape(DC, 128, 4, NB).transpose(2, 1, 0, 3).reshape(4 * 128, DC * NB)
    ).astype(bf16)
    wg_p = np.ascontiguousarray(
        np.asarray(Wg).reshape(DC, 128, H).transpose(1, 0, 2).reshape(128, DC * H)
    ).astype(bf16)
    wo_p = np.ascontiguousarray(
        np.asarray(Wo).reshape(H, 128, D).transpose(1, 0, 2).reshape(128, H * D)
    ).astype(bf16)
    bg_2 = np.ascontiguousarray(np.asarray(bg).reshape(H, 1)).astype(np.float32)

    in_maps = []
    for c in range(NCORES):
        b, r = divmod(c, 4)
        sl = slice(r * NB, (r + 1) * NB)
        xt_p = np.ascontiguousarray(
            np.asarray(x[b, sl, :]).reshape(NB, DC, 128).transpose(2, 1, 0).reshape(128, DC * NB)
        ).astype(bf16)
        in_maps.append({
            "xt": xt_p,
            "cos": np.ascontiguousarray(cosT[:, sl]).astype(bf16),
            "sins": np.ascontiguousarray(sinTs[:, sl]).astype(bf16),
            "wq": wq_p, "wk": wk_p, "wv": wv_p, "wg": wg_p,
            "bg": bg_2, "wo": wo_p,
        })
    return in_maps


def run(x, rotary_pos_emb, Wq, Wk, Wv, Wg, bg, Wo, trace=False):
    from concourse.bass_utils import run_bass_kernel_spmd

    nc = _get_nc()
    in_maps = _prep_in_maps(x, rotary_pos_emb, Wq, Wk, Wv, Wg, bg, Wo)
    kwargs = {}
    if trace:
        kwargs = dict(trace=True, trace_cores=list(range(NCORES)))
    res = run_bass_kernel_spmd(nc, in_maps, core_ids=list(range(NCORES)), **kwargs)
    full = np.empty((B, N, D), dtype=np.float32)
    for c in range(NCORES):
        b, r = divmod(c, 4)
        full[b, r * NB:(r + 1) * NB, :] = res.results[c]["out"]
    return full, res


def kernel(x, rotary_pos_emb, Wq, Wk, Wv, Wg, bg, Wo):
    full, _ = run(x, rotary_pos_emb, Wq, Wk, Wv, Wg, bg, Wo)
    return full
